# revision 3
# baseline (speedup 1.0000x reference)
"""Bass/Trainium2 kernel for nn_DWAMiddleLayer (low-rank MoE weight-assembly).

Math:
    t[b,n,r]  = sum_a V[n,r,a] h_A[b,a]
    s[b,n,r]  = alpha[b,n] * t[b,n,r]
    h_t[b,c]  = sum_{n,r} s[b,n,r] U[n,c,r] + alpha@bE + h_A@W_base^T + b_base
    y = h_A + gamma*h_t ; out = LN(y)*ln_scale + ln_bias

Strategy: data-parallel over batch (BS=256/core), pool replicated, all matmul
operands fp8 (host-side cast+scale as in v1). v2 changes vs the 26.7us v1:
  * DMA: 5 transfers balanced over both HWDGE queues (SP + ACT), ordered so
    the o=0 pipeline inputs (hAT, VT_o0, alT, U_o0) land first.  v1 serialized
    950KB behind one queue and starved the PE until ~13us.
  * PE duty-cycle (HAM) warmup: the PE powers up at 4/8 duty (213ns per
    256-col matmul) and reaches 8/8 (107ns) only after ~3.4us of
    *uninterrupted* matmul activity.  Dummy matmuls run back-to-back from
    context entry until real data lands, so the real stream runs mostly warm.
  * PE order: all mm1 for o0/o1 first (DMA-gated), mm2 interleaved behind the
    alpha-multiplies, extras (W_base, eye-residual, bias) mid-stream when
    their (later) transfers land, bch0's last accumulation closes before
    bch1's so the LN epilogue overlaps the final matmuls.
  * alpha-multiply (the serial DVE chain, 8 x 512cols x 1.04ns): two chunks
    offloaded to GpSimd (Pool) so the DVE chain shortens to ~4us.
  * Epilogue split: bn_stats/recip/apply(bch0) on DVE, sqrt + apply(bch1) on
    ACT (Identity with per-partition scale=rstd, bias=-mu*rstd), outputs on
    both queues in parallel.
LN is scale-invariant so ht is normalized directly (eps' = eps/g_eff^2).
"""

import numpy as np

B, N, D_A, D_B, R = 2048, 512, 256, 256, 4
NC_COUNT = 8
BS = B // NC_COUNT  # 256
P = 128
LN_EPS = 1e-5

N_DUMMY = 11        # PE warmup matmuls (213ns each cold) before data lands
PL_CHUNKS = ()      # GpSimd cannot read PSUM; TT offload disabled

# d_dve fp32-word layout (per partition)
EPS_OFF = 0    # eps/gamma_eff^2 fp32 [1]
EYE_OFF = 1    # eye128 bf16 [128] = 64 words
HAS_OFF = 65   # hAs bf16 [2,256] = 256 words
BE_OFF = 321   # bE fp8 [2,2,256] = 256 words
EP_OFF = 577   # ep bf16 [2,256] = 256 words (generic only)
DVE_W_TRIV = 577
DVE_W_GEN = 833

_cache = {}


def _build_nc(trivial_ep: bool):
    import concourse.mybir as mybir
    import concourse.tile as tile
    from concourse import bacc

    fp32 = mybir.dt.float32
    bf16 = mybir.dt.bfloat16
    f8 = mybir.dt.float8e4
    DR = mybir.MatmulPerfMode.DoubleRow
    DRSW = mybir.MatmulPerfMode.DoubleRowSwInterleave

    nc = bacc.Bacc("TRN2", target_bir_lowering=False)

    dve_w = DVE_W_TRIV if trivial_ep else DVE_W_GEN
    # inputs (f8 payloads packed per-partition; see make_in_maps)
    d_qs1 = nc.dram_tensor("qs1", [P, 2560], f8, kind="ExternalInput")  # hAT|VT0|VT1
    d_qs2 = nc.dram_tensor("qs2", [P, 3584], f8, kind="ExternalInput")  # VT2|VT3|U3|Wb
    d_qa1 = nc.dram_tensor("qa1", [P, 2048], f8, kind="ExternalInput")  # alT|U0
    d_qa2 = nc.dram_tensor("qa2", [P, 2048], f8, kind="ExternalInput")  # U1|U2
    d_dve = nc.dram_tensor("dve", [P, dve_w], fp32, kind="ExternalInput")
    d_out = nc.dram_tensor("out", [BS, D_A], fp32, kind="ExternalOutput")

    with tile.TileContext(nc) as tc:
        with (
            tc.tile_pool(name="persist", bufs=1) as persist,
            tc.tile_pool(name="spool", bufs=4) as spool,
            tc.tile_pool(name="sm", bufs=2) as sm,
            tc.tile_pool(name="pt", bufs=4, space="PSUM") as pt,
            tc.tile_pool(name="pacc", bufs=1, space="PSUM") as pacc,
            tc.tile_pool(name="pw", bufs=1, space="PSUM") as pw,
        ):
            # ---- DMA issues first: 2 on SP queue, 3 on ACT queue ----
            qs1 = persist.tile([P, 2560], f8)
            nc.sync.dma_start(qs1, d_qs1[:])
            qa1 = persist.tile([P, 2048], f8)
            nc.scalar.dma_start(qa1, d_qa1[:])
            qs2 = persist.tile([P, 3584], f8)
            nc.sync.dma_start(qs2, d_qs2[:])
            qa2 = persist.tile([P, 2048], f8)
            nc.scalar.dma_start(qa2, d_qa2[:])
            dvt = persist.tile([P, dve_w], fp32)
            nc.scalar.dma_start(dvt, d_dve[:])

            # ---- small consts + PE warmup source (GpSimd memsets) ----
            eps_col = persist.tile([P, 1], fp32)
            nc.gpsimd.memset(eps_col, LN_EPS)
            wz = persist.tile([P, 384], bf16)
            nc.gpsimd.memset(wz, 0.0)

            # ACT table preload happens before this first activation; it runs
            # during the DMA window so the epilogue Sqrt hits a warm table.
            warm = sm.tile([P, 1], fp32, tag="warm")
            nc.scalar.activation(
                warm, eps_col, mybir.ActivationFunctionType.Sqrt, bias=eps_col
            )

            # ---- views ----
            hAT = qs1[:, 0:512].rearrange("p (i b) -> p i b", i=2)  # [P,2,256]
            alT = qa1[:, 0:1024].rearrange("p (o b) -> p o b", o=4)  # [P,4,256]

            def vt_blk(o, r):  # mm1 lhsT block [P, 256] (hybrid DRSW layout)
                base = [qs1, qs1, qs2, qs2][o]
                off = [512, 1536, 0, 1024][o] + r * 256
                return base[:, off : off + 256].rearrange("p (j i) -> p j i", i=2)

            def u_blk(o, rp):  # mm2 rhs [P, 2, 256]
                base = [qa1, qa2, qa2, qs2][o]
                off = [1024, 0, 1024, 2048][o] + rp * 512
                return base[:, off : off + 512].rearrange("p (i c) -> p i c", i=2)

            Wb = qs2[:, 3072:3584].rearrange("p (i c) -> p i c", i=2)  # [P,2,256]

            epsp = dvt[:, EPS_OFF : EPS_OFF + 1]
            eye_b = dvt[:, EYE_OFF : EYE_OFF + 64].bitcast(bf16)  # [P,128]
            hAs = dvt[:, HAS_OFF : HAS_OFF + 256].bitcast(bf16).rearrange(
                "p (k c) -> p k c", k=2
            )
            bE = dvt[:, BE_OFF : BE_OFF + 256].bitcast(f8).rearrange(
                "p (op i c) -> p op i c", op=2, i=2
            )
            if not trivial_ep:
                ep = dvt[:, EP_OFF : EP_OFF + 256].bitcast(bf16).rearrange(
                    "p (k c) -> p k c", k=2
                )

            # ---- PE HAM warmup: back-to-back dummy matmuls during DMA wait ----
            pwt = pw.tile([P, 256], fp32)
            for _ in range(N_DUMMY):
                nc.tensor.matmul(
                    pwt,
                    lhsT=wz[:, 0:128],
                    rhs=wz[:, 128:384],
                    start=True,
                    stop=True,
                    skip_group_check=True,
                )

            # ---- ht accumulator [P, bch, c] ----
            ht = pacc.tile([P, 2, D_B], fp32)
            started = [False, False]

            def acc(bch, lhsT, rhs, pmode, last=False):
                nc.tensor.matmul(
                    ht[:, bch],
                    lhsT=lhsT,
                    rhs=rhs,
                    start=(not started[bch]),
                    stop=last,
                    perf_mode=pmode,
                    skip_group_check=True,
                )
                started[bch] = True

            # ---- main pipeline ----
            # chunk index k = o*2+rp; t_ps/s8 tiles per chunk
            t_ps = {}
            s8 = {}

            def mm1(o, rp):
                tp = pt.tile([P, 2, BS], fp32, tag="t")
                t_ps[(o, rp)] = tp
                for rr in range(2):
                    nc.tensor.matmul(
                        tp[:, rr],
                        lhsT=vt_blk(o, rp * 2 + rr),
                        rhs=hAT,
                        start=True,
                        stop=True,
                        perf_mode=DRSW,
                    )

            def tt(o, rp):  # alpha-multiply on DVE or GpSimd
                k = o * 2 + rp
                s = spool.tile([P, 2, BS], f8, tag="s")
                s8[(o, rp)] = s
                eng = nc.gpsimd if k in PL_CHUNKS else nc.vector
                eng.tensor_mul(
                    s, t_ps[(o, rp)], alT[:, o : o + 1, :].to_broadcast((P, 2, BS))
                )

            def mm2(o, rp, bchs=(0, 1), last=False):
                for bch in bchs:
                    lhsT = s8[(o, rp)][:, :, bch * P : (bch + 1) * P]
                    acc(bch, lhsT, u_blk(o, rp), DR, last=last)

            # o0/o1 mm1 first (gated by qs1), mm2+TT pipelined behind
            mm1(0, 0)
            mm1(0, 1)
            tt(0, 0)
            tt(0, 1)
            mm1(1, 0)
            mm1(1, 1)
            mm2(0, 0)
            mm2(0, 1)
            tt(1, 0)
            tt(1, 1)
            # o2/o3 mm1 (gated by qs2)
            mm1(2, 0)
            mm1(2, 1)
            mm2(1, 0)
            mm2(1, 1)
            tt(2, 0)
            tt(2, 1)
            mm1(3, 0)
            mm1(3, 1)
            # extras once their (late) transfers land: base(Wb qs2),
            # eye-residual + bias (dvt)
            for bch in range(2):
                b_lhsT = hAT[:, :, bch * P : (bch + 1) * P]
                acc(bch, b_lhsT, Wb, DR)
                nc.tensor.matmul(
                    ht[:, bch],
                    lhsT=eye_b,
                    rhs=hAs[:, bch],
                    start=False,
                    stop=False,
                    skip_group_check=True,
                )
            for op in range(2):
                for bch in range(2):
                    a_lhsT = alT[:, op * 2 : (op + 1) * 2, bch * P : (bch + 1) * P]
                    acc(bch, a_lhsT, bE[:, op], DR)
            mm2(2, 0)
            mm2(2, 1)
            tt(3, 0)
            tt(3, 1)
            mm2(3, 0)
            # final chunk: close bch0 before bch1 so its LN overlaps
            mm2(3, 1, bchs=(0,), last=True)
            mm2(3, 1, bchs=(1,), last=True)

            # ---- epilogue: LN is scale-invariant, normalize ht directly
            # (y = g*ht + resid with resid already inside ht via the eye-mm;
            #  (y-mu_y)*rsqrt(var_y+eps) == (ht-mu_ht)*rsqrt(var_ht+eps/g^2))
            stats = sm.tile([P, 2, 6], fp32, tag="st")
            mv = sm.tile([P, 2, 2], fp32, tag="mv")
            for bch in range(2):
                nc.vector.bn_stats(stats[:, bch], ht[:, bch])
                nc.vector.bn_aggr(mv[:, bch], stats[:, bch])
            rstd = sm.tile([P, 2], fp32, tag="rstd")
            nc.scalar.activation(
                rstd, mv[:, :, 1], mybir.ActivationFunctionType.Sqrt, bias=epsp
            )
            nc.vector.reciprocal(rstd, rstd)
            out_sb = sm.tile([P, 2, D_A], fp32, tag="out")

            if trivial_ep:
                # bch1 on ACT: out = Identity(ht*rstd + (-mu*rstd))
                nmr = sm.tile([P, 2], fp32, tag="nmr")
                nc.vector.tensor_scalar(
                    nmr[:, 1:2],
                    mv[:, 1, 0:1],
                    scalar1=rstd[:, 1:2],
                    scalar2=-1.0,
                    op0=mybir.AluOpType.mult,
                    op1=mybir.AluOpType.mult,
                )
                # bch0 on DVE
                nc.vector.tensor_scalar(
                    out_sb[:, 0],
                    ht[:, 0],
                    scalar1=mv[:, 0, 0:1],
                    scalar2=rstd[:, 0:1],
                    op0=mybir.AluOpType.subtract,
                    op1=mybir.AluOpType.mult,
                )
                nc.sync.dma_start(d_out[0:P, :], out_sb[:, 0])
                nc.scalar.activation(
                    out_sb[:, 1],
                    ht[:, 1],
                    mybir.ActivationFunctionType.Identity,
                    bias=nmr[:, 1:2],
                    scale=rstd[:, 1:2],
                )
                nc.scalar.dma_start(d_out[P : 2 * P, :], out_sb[:, 1])
            else:
                for bch in range(2):
                    nc.vector.tensor_scalar(
                        out_sb[:, bch],
                        ht[:, bch],
                        scalar1=mv[:, bch, 0:1],
                        scalar2=rstd[:, bch : bch + 1],
                        op0=mybir.AluOpType.subtract,
                        op1=mybir.AluOpType.mult,
                    )
                    nc.vector.tensor_mul(
                        out_sb[:, bch],
                        out_sb[:, bch],
                        ep[:, 0:1, :].rearrange("p u c -> p (u c)").to_broadcast((P, D_A)),
                    )
                    nc.vector.tensor_add(
                        out_sb[:, bch],
                        out_sb[:, bch],
                        ep[:, 1:2, :].rearrange("p u c -> p (u c)").to_broadcast((P, D_A)),
                    )
                    q = nc.sync if bch == 0 else nc.scalar
                    q.dma_start(d_out[bch * P : (bch + 1) * P, :], out_sb[:, bch])

    nc.compile()
    return nc


def _get_nc(trivial_ep):
    if trivial_ep not in _cache:
        _cache[trivial_ep] = _build_nc(trivial_ep)
    return _cache[trivial_ep]


def make_in_maps(trivial_ep, **inputs):
    import ml_dtypes

    f8 = ml_dtypes.float8_e4m3
    q8 = lambda x: np.clip(x, -240, 240).astype(f8)

    f32 = lambda k: np.asarray(inputs[k], np.float32)
    h_A = f32("h_A")
    pool = f32("pool_vectors")
    alpha = f32("alpha")
    W_base = f32("W_base")
    b_base = f32("b_base").reshape(D_B)
    gamma = float(np.asarray(inputs["gamma"]).reshape(()))
    ln_s = f32("ln_scale").reshape(D_A)
    ln_b = f32("ln_bias").reshape(D_A)

    U = pool[:, : D_B * R].reshape(N, D_B, R)
    V = pool[:, D_B * R : D_B * R + R * D_A].reshape(N, R, D_A)
    bE = pool[:, D_B * R + R * D_A : D_B * R + R * D_A + D_B]

    V8 = q8(V * 16.0)  # [n, r, a]
    U8 = q8(U * 16.0)  # [n, c, r]
    bE8 = q8(bE * 256.0)  # [n, c]
    Wb8 = q8(W_base * 256.0)  # [c, a]
    g_eff = gamma / 256.0

    # ---- shared (pool-side) packing ----
    # VT blocks [P, o, r, 256]  (hybrid layout: [p, i, m])
    VTb = np.empty((P, 4, 4, 256), f8)
    V8v = V8.reshape(4, P, R, 2, P)  # [o, n, r, i, p]
    for o in range(4):
        for r in range(R):
            blk = V8v[o, :, r]  # [n=128(m), i, p]
            VTb[:, o, r] = blk.transpose(2, 1, 0).reshape(P, 256)  # p, i, m
    # U mm2-rhs [p, o, rp, rr, c]   (U8.reshape dims = (o, n_p, c, rp, rr))
    Ub = np.ascontiguousarray(U8.reshape(4, P, D_B, 2, 2).transpose(1, 0, 3, 4, 2))
    bEb = np.ascontiguousarray(
        bE8.reshape(2, 2, P, D_B).transpose(2, 0, 1, 3)
    )  # [p, op, i, c]
    Wbb = np.ascontiguousarray(
        Wb8.reshape(D_B, 2, P).transpose(2, 1, 0)
    )  # [p, i, c]

    qs2 = np.empty((P, 3584), f8)
    qs2[:, :1024] = VTb[:, 2].reshape(P, 1024)
    qs2[:, 1024:2048] = VTb[:, 3].reshape(P, 1024)
    qs2[:, 2048:3072] = Ub[:, 3].reshape(P, 1024)
    qs2[:, 3072:] = Wbb.reshape(P, 512)
    qa2 = np.empty((P, 2048), f8)
    qa2[:, :1024] = Ub[:, 1].reshape(P, 1024)
    qa2[:, 1024:] = Ub[:, 2].reshape(P, 1024)

    eye_words = (
        np.eye(P, dtype=np.float32).astype(ml_dtypes.bfloat16).view(np.float32)
    )  # [P, 64]

    dve_w = DVE_W_TRIV if trivial_ep else DVE_W_GEN
    in_maps = []
    for ci in range(NC_COUNT):
        sl = slice(ci * BS, (ci + 1) * BS)
        hA_c = h_A[sl]  # [256, 256]
        al_c = alpha[sl]  # [256, 512]
        hA8 = q8(hA_c)  # [b, a]
        al8 = q8(al_c)

        qs1 = np.empty((P, 2560), f8)
        # hAT [p, i, b] = hA8[b, i*128+p]
        qs1[:, :512] = hA8.reshape(BS, 2, P).transpose(2, 1, 0).reshape(P, 512)
        qs1[:, 512:1536] = VTb[:, 0].reshape(P, 1024)
        qs1[:, 1536:] = VTb[:, 1].reshape(P, 1024)

        qa1 = np.empty((P, 2048), f8)
        # alT [p, o, b] = al8[b, o*128+p]
        qa1[:, :1024] = al8.reshape(BS, 4, P).transpose(2, 1, 0).reshape(P, 1024)
        qa1[:, 1024:] = Ub[:, 0].reshape(P, 1024)

        dve = np.zeros((P, dve_w), np.float32)
        dve[:, EPS_OFF] = LN_EPS / (g_eff * g_eff)
        dve[:, EYE_OFF : EYE_OFF + 64] = eye_words
        # hAs [p, bch, c] = (h_A[b(p,bch)] + gamma*b_base) / g_eff, bf16
        hAs_rows = (hA_c + gamma * b_base[None, :]) / g_eff
        hAs = hAs_rows.reshape(2, P, D_A)  # [bch, m, c] row index = b%128
        dve[:, HAS_OFF : HAS_OFF + 256] = (
            hAs.transpose(1, 0, 2).reshape(P, 512).astype(ml_dtypes.bfloat16)
        ).view(np.float32)
        dve[:, BE_OFF : BE_OFF + 256] = bEb.reshape(P, 1024).view(np.float32)
        if not trivial_ep:
            epb = np.empty((2, D_A), np.float32)
            epb[0] = ln_s
            epb[1] = ln_b
            dve[:, EP_OFF : EP_OFF + 256] = np.broadcast_to(
                epb.reshape(1, 512), (P, 512)
            ).astype(ml_dtypes.bfloat16).view(np.float32)

        in_maps.append(
            {"qs1": qs1, "qs2": qs2, "qa1": qa1, "qa2": qa2, "dve": dve}
        )
    return in_maps


def run_kernel(trace=False, **inputs):
    from concourse.bass_utils import run_bass_kernel_spmd

    ln_s = np.asarray(inputs["ln_scale"], np.float32)
    ln_b = np.asarray(inputs["ln_bias"], np.float32)
    trivial_ep = bool(np.all(ln_s == 1.0) and np.all(ln_b == 0.0))
    nc = _get_nc(trivial_ep)
    in_maps = make_in_maps(trivial_ep, **inputs)
    res = run_bass_kernel_spmd(nc, in_maps, core_ids=list(range(NC_COUNT)), trace=trace)
    outs = [r["out"] for r in res.results]
    out = np.concatenate(outs, axis=0)
    return np.ascontiguousarray(out).astype(np.float32), res


def kernel(**inputs) -> np.ndarray:
    out, _ = run_kernel(trace=False, **inputs)
    return out


# revision 13
# speedup vs baseline: 1.1238x; 1.1238x over previous
"""Bass/Trainium2 kernel for nn_DWAMiddleLayer (low-rank MoE weight-assembly).

Math:
    t[b,n,r]  = sum_a V[n,r,a] h_A[b,a]
    s[b,n,r]  = alpha[b,n] * t[b,n,r]
    h_t[b,c]  = sum_{n,r} s[b,n,r] U[n,c,r] + alpha@bE + h_A@W_base^T + b_base
    y = h_A + gamma*h_t ; out = LN(y)*ln_scale + ln_bias

Strategy: data-parallel over batch (BS=256/core), pool replicated, all matmul
operands fp8 (host-side cast+scale as in v1). v2 changes vs the 26.7us v1:
  * DMA: 5 transfers balanced over both HWDGE queues (SP + ACT), ordered so
    the o=0 pipeline inputs (hAT, VT_o0, alT, U_o0) land first.  v1 serialized
    950KB behind one queue and starved the PE until ~13us.
  * PE duty-cycle (HAM) warmup: the PE powers up at 4/8 duty (213ns per
    256-col matmul) and reaches 8/8 (107ns) only after ~3.4us of
    *uninterrupted* matmul activity.  Dummy matmuls run back-to-back from
    context entry until real data lands, so the real stream runs mostly warm.
  * PE order: all mm1 for o0/o1 first (DMA-gated), mm2 interleaved behind the
    alpha-multiplies, extras (W_base, eye-residual, bias) mid-stream when
    their (later) transfers land, bch0's last accumulation closes before
    bch1's so the LN epilogue overlaps the final matmuls.
  * alpha-multiply (the serial DVE chain, 8 x 512cols x 1.04ns): two chunks
    offloaded to GpSimd (Pool) so the DVE chain shortens to ~4us.
  * Epilogue split: bn_stats/recip/apply(bch0) on DVE, sqrt + apply(bch1) on
    ACT (Identity with per-partition scale=rstd, bias=-mu*rstd), outputs on
    both queues in parallel.
LN is scale-invariant so ht is normalized directly (eps' = eps/g_eff^2).
"""

import numpy as np

B, N, D_A, D_B, R = 2048, 512, 256, 256, 4
NC_COUNT = 8
BS = B // NC_COUNT  # 256
P = 128
LN_EPS = 1e-5

N_DUMMY = 10        # PE warmup matmuls (213ns each cold) before data lands

# d_dve fp32-word layout (per partition)
EPS_OFF = 0    # eps/gamma_eff^2 fp32 [1]
EYE_OFF = 1    # eye128 bf16 [128] = 64 words
HAS_OFF = 65   # hAs bf16 [2,256] = 256 words
BE_OFF = 321   # bE fp8 [2,2,256] = 256 words
WB_OFF = 577   # Wb fp8 [2,256] = 128 words
EP_OFF = 705   # ep bf16 [2,256] = 256 words (generic only)
DVE_W_TRIV = 705
DVE_W_GEN = 961

_cache = {}


def _build_nc(trivial_ep: bool):
    import concourse.mybir as mybir
    import concourse.tile as tile
    from concourse import bacc

    fp32 = mybir.dt.float32
    bf16 = mybir.dt.bfloat16
    f8 = mybir.dt.float8e4
    DR = mybir.MatmulPerfMode.DoubleRow
    DRSW = mybir.MatmulPerfMode.DoubleRowSwInterleave

    nc = bacc.Bacc("TRN2", target_bir_lowering=False)

    dve_w = DVE_W_TRIV if trivial_ep else DVE_W_GEN
    # inputs (f8 payloads packed per-partition; see make_in_maps).  The two
    # HWDGE queues share one ~300 B/ns engine pool, so transfers are strictly
    # priority-ordered: SP streams the TT-chain inputs (hAT, alT, VT) plus
    # U_o0 back-to-back; the ACT queue's transfers are held back by a
    # cross-engine dependency bridge until the critical prefix has landed.
    d_qs1 = nc.dram_tensor("qs1", [P, 2560], f8, kind="ExternalInput")  # hAT|alT|VT0
    d_qs2 = nc.dram_tensor("qs2", [P, 2048], f8, kind="ExternalInput")  # U0|VT1
    d_qs3 = nc.dram_tensor("qs3", [P, 2048], f8, kind="ExternalInput")  # VT2|VT3
    d_qa1 = nc.dram_tensor("qa1", [P, 3072], f8, kind="ExternalInput")  # U1|U2|U3
    d_dve = nc.dram_tensor("dve", [P, dve_w], fp32, kind="ExternalInput")
    d_out = nc.dram_tensor("out", [BS, D_A], fp32, kind="ExternalOutput")

    with tile.TileContext(nc) as tc:
        with (
            tc.tile_pool(name="persist", bufs=1) as persist,
            tc.tile_pool(name="spool", bufs=4) as spool,
            tc.tile_pool(name="sm", bufs=2) as sm,
            tc.tile_pool(name="pt", bufs=4, space="PSUM") as pt,
            tc.tile_pool(name="pacc", bufs=1, space="PSUM") as pacc,
            tc.tile_pool(name="pw", bufs=1, space="PSUM") as pw,
        ):
            # ---- SP queue: critical stream, 3 priority-ordered transfers ----
            qs1 = persist.tile([P, 2560], f8)
            nc.sync.dma_start(qs1, d_qs1[:])
            qs2 = persist.tile([P, 2048], f8)
            nc.sync.dma_start(qs2, d_qs2[:])
            qs3 = persist.tile([P, 2048], f8)
            nc.sync.dma_start(qs3, d_qs3[:])

            # ---- PE warmup source + small consts (GpSimd memsets) ----
            wz = persist.tile([P, 384], bf16)
            nc.gpsimd.memset(wz, 0.0)
            eps_col = persist.tile([P, 1], fp32)
            nc.gpsimd.memset(eps_col, LN_EPS)

            # ---- ACT queue: held back until qs1 lands (bridge via GpSimd
            # write into the dest tile -> the dma_start gains a WAW wait) ----
            qa1 = persist.tile([P, 3072], f8)
            nc.gpsimd.tensor_copy(qa1[:, 0:1], qs1[:, 0:1])
            nc.scalar.dma_start(qa1, d_qa1[:])
            dvt = persist.tile([P, dve_w], fp32)
            nc.scalar.dma_start(dvt, d_dve[:])

            # ACT table preload happens before this first activation; it runs
            # during the DMA window so the epilogue Sqrt hits a warm table.
            warm = sm.tile([P, 1], fp32, tag="warm")
            nc.scalar.activation(
                warm, eps_col, mybir.ActivationFunctionType.Sqrt, bias=eps_col
            )

            # ---- views ----
            hAT = qs1[:, 0:512].rearrange("p (i b) -> p i b", i=2)  # [P,2,256]
            alT = qs1[:, 512:1536].rearrange("p (o b) -> p o b", o=4)  # [P,4,256]

            def vt_blk(o, r):  # mm1 lhsT block [P, 256] (hybrid DRSW layout)
                base = [qs1, qs2, qs3, qs3][o]
                off = [1536, 1024, 0, 1024][o] + r * 256
                return base[:, off : off + 256].rearrange("p (j i) -> p j i", i=2)

            def u_blk(o, rp):  # mm2 rhs [P, 2, 256]
                base = [qs2, qa1, qa1, qa1][o]
                off = [0, 0, 1024, 2048][o] + rp * 512
                return base[:, off : off + 512].rearrange("p (i c) -> p i c", i=2)

            Wb = dvt[:, WB_OFF : WB_OFF + 128].bitcast(f8).rearrange(
                "p (i c) -> p i c", i=2
            )  # [P,2,256]

            epsp = dvt[:, EPS_OFF : EPS_OFF + 1]
            eye_b = dvt[:, EYE_OFF : EYE_OFF + 64].bitcast(bf16)  # [P,128]
            hAs = dvt[:, HAS_OFF : HAS_OFF + 256].bitcast(bf16).rearrange(
                "p (k c) -> p k c", k=2
            )
            bE = dvt[:, BE_OFF : BE_OFF + 256].bitcast(f8).rearrange(
                "p (op i c) -> p op i c", op=2, i=2
            )
            if not trivial_ep:
                ep = dvt[:, EP_OFF : EP_OFF + 256].bitcast(bf16).rearrange(
                    "p (k c) -> p k c", k=2
                )

            # ---- PE HAM warmup: back-to-back dummy matmuls during DMA wait ----
            pwt = pw.tile([P, 256], fp32)
            for _ in range(N_DUMMY):
                nc.tensor.matmul(
                    pwt,
                    lhsT=wz[:, 0:128],
                    rhs=wz[:, 128:384],
                    start=True,
                    stop=True,
                    skip_group_check=True,
                )

            # ---- ht accumulator [P, bch, c] ----
            ht = pacc.tile([P, 2, D_B], fp32)
            started = [False, False]

            def acc(bch, lhsT, rhs, pmode, last=False):
                nc.tensor.matmul(
                    ht[:, bch],
                    lhsT=lhsT,
                    rhs=rhs,
                    start=(not started[bch]),
                    stop=last,
                    perf_mode=pmode,
                    skip_group_check=True,
                )
                started[bch] = True

            # ---- main pipeline ----
            # chunk index k = o*2+rp; t_ps/s8 tiles per chunk
            t_ps = {}
            s8 = {}

            def mm1(o, rp):
                tp = pt.tile([P, 2, BS], fp32, tag="t")
                t_ps[(o, rp)] = tp
                for rr in range(2):
                    nc.tensor.matmul(
                        tp[:, rr],
                        lhsT=vt_blk(o, rp * 2 + rr),
                        rhs=hAT,
                        start=True,
                        stop=True,
                        perf_mode=DRSW,
                    )

            def tt(o, rp):  # alpha-multiply (the serial DVE chain)
                s = spool.tile([P, 2, BS], f8, tag="s")
                s8[(o, rp)] = s
                nc.vector.tensor_mul(
                    s, t_ps[(o, rp)], alT[:, o : o + 1, :].to_broadcast((P, 2, BS))
                )

            def mm2(o, rp, bchs=(0, 1), last=False):
                for bch in bchs:
                    lhsT = s8[(o, rp)][:, :, bch * P : (bch + 1) * P]
                    acc(bch, lhsT, u_blk(o, rp), DR, last=last)

            # o0 mm1 first (gated by qs1), rest pipelined behind the DMA
            # stream; TTs emitted right after their mm1 pair (DVE in-order).
            mm1(0, 0)
            mm1(0, 1)
            tt(0, 0)
            tt(0, 1)
            mm1(1, 0)  # qs2
            mm1(1, 1)
            tt(1, 0)
            tt(1, 1)
            mm2(0, 0)  # U_o0 in qs2
            mm2(0, 1)
            mm1(2, 0)  # qs3
            mm1(2, 1)
            tt(2, 0)
            tt(2, 1)
            mm1(3, 0)
            mm1(3, 1)
            tt(3, 0)
            tt(3, 1)
            mm2(1, 0)  # U_o1 in qa1
            mm2(1, 1)
            mm2(2, 0)
            # extras once the late dvt transfer lands: base(Wb), eye, bias
            for bch in range(2):
                b_lhsT = hAT[:, :, bch * P : (bch + 1) * P]
                acc(bch, b_lhsT, Wb, DR)
                nc.tensor.matmul(
                    ht[:, bch],
                    lhsT=eye_b,
                    rhs=hAs[:, bch],
                    start=False,
                    stop=False,
                    skip_group_check=True,
                )
            for op in range(2):
                for bch in range(2):
                    a_lhsT = alT[:, op * 2 : (op + 1) * 2, bch * P : (bch + 1) * P]
                    acc(bch, a_lhsT, bE[:, op], DR)
            mm2(2, 1)
            mm2(3, 0)
            # final chunk: close bch0 before bch1 so its LN overlaps
            mm2(3, 1, bchs=(0,), last=True)
            mm2(3, 1, bchs=(1,), last=True)

            # ---- epilogue: LN is scale-invariant, normalize ht directly
            # (y = g*ht + resid with resid already inside ht via the eye-mm;
            #  (y-mu_y)*rsqrt(var_y+eps) == (ht-mu_ht)*rsqrt(var_ht+eps/g^2))
            stats = sm.tile([P, 2, 6], fp32, tag="st")
            mv = sm.tile([P, 2, 2], fp32, tag="mv")
            for bch in range(2):
                nc.vector.bn_stats(stats[:, bch], ht[:, bch])
                nc.vector.bn_aggr(mv[:, bch], stats[:, bch])
            rstd = sm.tile([P, 2], fp32, tag="rstd")
            nc.scalar.activation(
                rstd, mv[:, :, 1], mybir.ActivationFunctionType.Sqrt, bias=epsp
            )
            nc.vector.reciprocal(rstd, rstd)
            out_sb = sm.tile([P, 2, D_A], fp32, tag="out")

            if trivial_ep:
                # bch1 on ACT: out = Identity(ht*rstd + (-mu*rstd))
                nmr = sm.tile([P, 2], fp32, tag="nmr")
                nc.vector.tensor_scalar(
                    nmr[:, 1:2],
                    mv[:, 1, 0:1],
                    scalar1=rstd[:, 1:2],
                    scalar2=-1.0,
                    op0=mybir.AluOpType.mult,
                    op1=mybir.AluOpType.mult,
                )
                # bch0 on DVE
                nc.vector.tensor_scalar(
                    out_sb[:, 0],
                    ht[:, 0],
                    scalar1=mv[:, 0, 0:1],
                    scalar2=rstd[:, 0:1],
                    op0=mybir.AluOpType.subtract,
                    op1=mybir.AluOpType.mult,
                )
                nc.sync.dma_start(d_out[0:P, :], out_sb[:, 0])
                nc.scalar.activation(
                    out_sb[:, 1],
                    ht[:, 1],
                    mybir.ActivationFunctionType.Identity,
                    bias=nmr[:, 1:2],
                    scale=rstd[:, 1:2],
                )
                nc.scalar.dma_start(d_out[P : 2 * P, :], out_sb[:, 1])
            else:
                for bch in range(2):
                    nc.vector.tensor_scalar(
                        out_sb[:, bch],
                        ht[:, bch],
                        scalar1=mv[:, bch, 0:1],
                        scalar2=rstd[:, bch : bch + 1],
                        op0=mybir.AluOpType.subtract,
                        op1=mybir.AluOpType.mult,
                    )
                    nc.vector.tensor_mul(
                        out_sb[:, bch],
                        out_sb[:, bch],
                        ep[:, 0:1, :].rearrange("p u c -> p (u c)").to_broadcast((P, D_A)),
                    )
                    nc.vector.tensor_add(
                        out_sb[:, bch],
                        out_sb[:, bch],
                        ep[:, 1:2, :].rearrange("p u c -> p (u c)").to_broadcast((P, D_A)),
                    )
                    q = nc.sync if bch == 0 else nc.scalar
                    q.dma_start(d_out[bch * P : (bch + 1) * P, :], out_sb[:, bch])

    nc.compile()
    return nc


def _get_nc(trivial_ep):
    if trivial_ep not in _cache:
        _cache[trivial_ep] = _build_nc(trivial_ep)
    return _cache[trivial_ep]


def make_in_maps(trivial_ep, **inputs):
    import ml_dtypes

    f8 = ml_dtypes.float8_e4m3
    q8 = lambda x: np.clip(x, -240, 240).astype(f8)

    f32 = lambda k: np.asarray(inputs[k], np.float32)
    h_A = f32("h_A")
    pool = f32("pool_vectors")
    alpha = f32("alpha")
    W_base = f32("W_base")
    b_base = f32("b_base").reshape(D_B)
    gamma = float(np.asarray(inputs["gamma"]).reshape(()))
    ln_s = f32("ln_scale").reshape(D_A)
    ln_b = f32("ln_bias").reshape(D_A)

    U = pool[:, : D_B * R].reshape(N, D_B, R)
    V = pool[:, D_B * R : D_B * R + R * D_A].reshape(N, R, D_A)
    bE = pool[:, D_B * R + R * D_A : D_B * R + R * D_A + D_B]

    V8 = q8(V * 16.0)  # [n, r, a]
    U8 = q8(U * 16.0)  # [n, c, r]
    bE8 = q8(bE * 256.0)  # [n, c]
    Wb8 = q8(W_base * 256.0)  # [c, a]
    g_eff = gamma / 256.0

    # ---- shared (pool-side) packing ----
    # VT blocks [P, o, r, 256]  (hybrid layout: [p, i, m])
    VTb = np.empty((P, 4, 4, 256), f8)
    V8v = V8.reshape(4, P, R, 2, P)  # [o, n, r, i, p]
    for o in range(4):
        for r in range(R):
            blk = V8v[o, :, r]  # [n=128(m), i, p]
            VTb[:, o, r] = blk.transpose(2, 1, 0).reshape(P, 256)  # p, i, m
    # U mm2-rhs [p, o, rp, rr, c]   (U8.reshape dims = (o, n_p, c, rp, rr))
    Ub = np.ascontiguousarray(U8.reshape(4, P, D_B, 2, 2).transpose(1, 0, 3, 4, 2))
    bEb = np.ascontiguousarray(
        bE8.reshape(2, 2, P, D_B).transpose(2, 0, 1, 3)
    )  # [p, op, i, c]
    Wbb = np.ascontiguousarray(
        Wb8.reshape(D_B, 2, P).transpose(2, 1, 0)
    )  # [p, i, c]

    qs3 = np.empty((P, 2048), f8)
    qs3[:, :1024] = VTb[:, 2].reshape(P, 1024)
    qs3[:, 1024:] = VTb[:, 3].reshape(P, 1024)
    qa1 = np.empty((P, 3072), f8)
    qa1[:, :1024] = Ub[:, 1].reshape(P, 1024)
    qa1[:, 1024:2048] = Ub[:, 2].reshape(P, 1024)
    qa1[:, 2048:] = Ub[:, 3].reshape(P, 1024)

    eye_words = (
        np.eye(P, dtype=np.float32).astype(ml_dtypes.bfloat16).view(np.float32)
    )  # [P, 64]

    dve_w = DVE_W_TRIV if trivial_ep else DVE_W_GEN
    in_maps = []
    for ci in range(NC_COUNT):
        sl = slice(ci * BS, (ci + 1) * BS)
        hA_c = h_A[sl]  # [256, 256]
        al_c = alpha[sl]  # [256, 512]
        hA8 = q8(hA_c)  # [b, a]
        al8 = q8(al_c)

        qs1 = np.empty((P, 2560), f8)
        # hAT [p, i, b] = hA8[b, i*128+p]
        qs1[:, :512] = hA8.reshape(BS, 2, P).transpose(2, 1, 0).reshape(P, 512)
        # alT [p, o, b] = al8[b, o*128+p]
        qs1[:, 512:1536] = al8.reshape(BS, 4, P).transpose(2, 1, 0).reshape(P, 1024)
        qs1[:, 1536:] = VTb[:, 0].reshape(P, 1024)

        qs2 = np.empty((P, 2048), f8)
        qs2[:, :1024] = Ub[:, 0].reshape(P, 1024)
        qs2[:, 1024:] = VTb[:, 1].reshape(P, 1024)

        dve = np.zeros((P, dve_w), np.float32)
        dve[:, EPS_OFF] = LN_EPS / (g_eff * g_eff)
        dve[:, EYE_OFF : EYE_OFF + 64] = eye_words
        # hAs [p, bch, c] = (h_A[b(p,bch)] + gamma*b_base) / g_eff, bf16
        hAs_rows = (hA_c + gamma * b_base[None, :]) / g_eff
        hAs = hAs_rows.reshape(2, P, D_A)  # [bch, m, c] row index = b%128
        dve[:, HAS_OFF : HAS_OFF + 256] = (
            hAs.transpose(1, 0, 2).reshape(P, 512).astype(ml_dtypes.bfloat16)
        ).view(np.float32)
        dve[:, BE_OFF : BE_OFF + 256] = bEb.reshape(P, 1024).view(np.float32)
        dve[:, WB_OFF : WB_OFF + 128] = Wbb.reshape(P, 512).view(np.float32)
        if not trivial_ep:
            epb = np.empty((2, D_A), np.float32)
            epb[0] = ln_s
            epb[1] = ln_b
            dve[:, EP_OFF : EP_OFF + 256] = np.broadcast_to(
                epb.reshape(1, 512), (P, 512)
            ).astype(ml_dtypes.bfloat16).view(np.float32)

        in_maps.append(
            {"qs1": qs1, "qs2": qs2, "qs3": qs3, "qa1": qa1, "dve": dve}
        )
    return in_maps


def run_kernel(trace=False, **inputs):
    from concourse.bass_utils import run_bass_kernel_spmd

    ln_s = np.asarray(inputs["ln_scale"], np.float32)
    ln_b = np.asarray(inputs["ln_bias"], np.float32)
    trivial_ep = bool(np.all(ln_s == 1.0) and np.all(ln_b == 0.0))
    nc = _get_nc(trivial_ep)
    in_maps = make_in_maps(trivial_ep, **inputs)
    res = run_bass_kernel_spmd(nc, in_maps, core_ids=list(range(NC_COUNT)), trace=trace)
    outs = [r["out"] for r in res.results]
    out = np.concatenate(outs, axis=0)
    return np.ascontiguousarray(out).astype(np.float32), res


def kernel(**inputs) -> np.ndarray:
    out, _ = run_kernel(trace=False, **inputs)
    return out
